# revision 12
# baseline (speedup 1.0000x reference)
"""Trainium2 Bass kernel: fp8-quantized Dense (8192x4096 @ 4096x16384) + bias + tanh-GELU.

Strategy (tensor-parallel over units, 8 cores), v2:
  - host: transpose x -> xT [d_in, tokens]; shard kernel/bias along units.
  - device per core:
      phase 1: amax scans (k shard 32 MiB, then this core's 1/8 token slice of
               xT 16 MiB) in 1 MiB chunks on both HWDGE rings; one
               AllReduce(max) carries [amax_k, amax_x]. CC input/readback DMAs
               ride the gpsimd SWDGE queue so they never block the rings.
      phase 2: scales via reciprocal + one Newton step (~1 ulp; the exact-RNE
               division of v1 isn't needed at the 2e-2 gate). Fused one-op
               quantizes: k on DVE (tensor_scalar mul -> fp8), x on ACT
               (activation Copy w/ scale -> fp8).
      phase 3: k shard restreamed as [128,2,1024] column-half chunks, units
               0..1023 first: block 0 runs kk-ordered in two 8-ub PSUM groups
               so its matmuls consume chunks in exactly the restream order --
               the PE starts ~195 us in and the restream hides behind it.
               x streams in 512-token blocks (f32, quantized on arrival into
               a double-buffered fp8 xq), ub-ordered DoubleRow matmuls,
               ACT epilogue gelu_tanh(psum * inv_scale + bias) -> f16 out.
  - fp8 numerics: quantize with 224/amax (half the reference's 448/amax, a
    power-of-two ratio) so the TRN fp8e4 grid matches OCP e4m3fn exactly in
    [-240, 240]; dequant amax_x*amax_k/224^2 restores the reference values.
  - output is produced transposed ([units, tokens] f16 per core); the host
    gathers shards, transposes, and upcasts to f32.
"""

import sys

sys.path.insert(0, "/opt/trn_rl_repo")

from contextlib import ExitStack

import numpy as np

import concourse.bacc as bacc
import concourse.tile as tile
from concourse import mybir
from concourse.bass_utils import run_bass_kernel_spmd

P = 128
FP8_HW_MAX = 224.0  # 448/2: keeps hw fp8 values inside TRN's +/-240 range

TOKENS, D_IN, UNITS, N_CORES = 8192, 4096, 16384, 8
US = UNITS // N_CORES          # 2048 units per core
KO_N = D_IN // P               # 32 d_in slabs
NPAIR = KO_N // 2              # 16 DoubleRow (256-contraction) steps
NU = US // P                   # 16 output unit-blocks
BLK = 512                      # token block
NBLK = TOKENS // BLK           # 16
AMX_T = TOKENS // N_CORES      # 1024 tokens scanned per core


def build(n_cores=N_CORES):
    dt = mybir.dt
    f32 = dt.float32
    f16 = dt.float16
    fp8 = dt.float8e4
    X = mybir.AxisListType.X
    MAX = mybir.AluOpType.max
    COPY = mybir.ActivationFunctionType.Copy
    GELU = mybir.ActivationFunctionType.Gelu_apprx_tanh
    DR = mybir.MatmulPerfMode.DoubleRow

    nc = bacc.Bacc("TRN2", target_bir_lowering=False, debug=False, num_devices=n_cores)
    xT = nc.dram_tensor("xT", [D_IN, TOKENS], f32, kind="ExternalInput").ap()
    xsl = nc.dram_tensor("xsl", [D_IN, AMX_T], f32, kind="ExternalInput").ap()
    ksh = nc.dram_tensor("ksh", [D_IN, US], f32, kind="ExternalInput").ap()
    bsh = nc.dram_tensor("bsh", [US], f32, kind="ExternalInput").ap()
    out = nc.dram_tensor("out", [US, TOKENS], f16, kind="ExternalOutput").ap()

    xTr = xT.rearrange("(n p) t -> p n t", p=P)    # [128, 32, 8192]
    xslr = xsl.rearrange("(n p) t -> p n t", p=P)  # [128, 32, 1024]
    kshr = ksh.rearrange("(n p) c -> p n c", p=P)  # [128, 32, 2048]

    from concourse.tile_rust import add_dep_helper
    from concourse import bass_isa

    with tile.TileContext(nc) as tc, ExitStack() as ctx:
        const = ctx.enter_context(tc.tile_pool(name="const", bufs=1))
        small = ctx.enter_context(tc.tile_pool(name="small", bufs=1))
        kqp = ctx.enter_context(tc.tile_pool(name="kqp", bufs=1))      # 64 KiB/part
        kst = ctx.enter_context(tc.tile_pool(name="kst", bufs=3))      # 3x8 KiB
        xpool = ctx.enter_context(tc.tile_pool(name="xpool", bufs=8))  # 8x8 KiB
        xqp = ctx.enter_context(tc.tile_pool(name="xqp", bufs=3))      # 3x16 KiB
        outp = ctx.enter_context(tc.tile_pool(name="outp", bufs=4))    # 4x1 KiB
        psum = ctx.enter_context(tc.tile_pool(name="psum", bufs=8, space="PSUM"))
        dram = ctx.enter_context(tc.tile_pool(name="dram", bufs=1, space="DRAM"))

        def ring(i):
            return nc.sync if i % 2 == 0 else nc.scalar

        # ---- bias shard, [P, NU]: bias_t[p, ub] = bias[ub*128 + p] ----
        bias_t = const.tile([P, NU], f32, name="bias_t")
        nc.sync.dma_start(bias_t[:], bsh.rearrange("(o p) -> p o", p=P))

        # ---- amax scans: 2 MiB chunks staged in the (idle) xq pool ----
        # k: 16 x [P,2,2048], x-slice: 8 x [P,4,1024], alternating HWDGE rings.
        rk = const.tile([P, NPAIR], f32, name="rk")
        last_scan = None
        for i in range(NPAIR):
            st = xqp.tile([P, 2, 2048], f32, tag="xq", name="kscan")
            last_scan = ring(i).dma_start(st[:], kshr[:, 2 * i : 2 * i + 2, :])
            nc.vector.tensor_reduce(
                rk[:, i : i + 1], st[:].rearrange("p a b -> p (a b)"), axis=X,
                op=MAX, apply_absolute_value=True,
            )
        last_kscan = last_scan

        rx = const.tile([P, 8], f32, name="rx")
        for i in range(8):
            st = xqp.tile([P, 4, 1024], f32, tag="xq", name="xscan")
            last_scan = ring(i).dma_start(st[:], xslr[:, 4 * i : 4 * i + 4, :])
            nc.vector.tensor_reduce(
                rx[:, i : i + 1], st[:].rearrange("p a b -> p (a b)"), axis=X,
                op=MAX, apply_absolute_value=True,
            )
        last_xscan = last_scan

        # ---- AllGather of per-core [amax_k, amax_x]; local max-combine ----
        colk = small.tile([P, 1], f32, name="colk")
        nc.vector.tensor_reduce(colk[:], rk[:], axis=X, op=MAX)
        nc.gpsimd.partition_all_reduce(colk[:], colk[:], P, bass_isa.ReduceOp.max)
        colx = small.tile([P, 1], f32, name="colx")
        nc.vector.tensor_reduce(colx[:], rx[:], axis=X, op=MAX)
        nc.gpsimd.partition_all_reduce(colx[:], colx[:], P, bass_isa.ReduceOp.max)

        pk2 = small.tile([1, 2], f32, name="pk2")
        nc.vector.tensor_copy(pk2[:, 0:1], colk[0:1, :])
        nc.vector.tensor_copy(pk2[:, 1:2], colx[0:1, :])
        cc_in = dram.tile([1, 2], f32, name="cc_in")
        nc.gpsimd.dma_start(cc_in[:], pk2[:])
        cc_out = dram.tile([1, 2 * n_cores], f32, name="cc_out", addr_space="Shared")
        nc.gpsimd.collective_compute(
            "AllGather", mybir.AluOpType.bypass,
            replica_groups=[list(range(n_cores))],
            ins=[cc_in[:].opt()], outs=[cc_out[:].opt()],
        )
        g16 = small.tile([1, 2 * n_cores], f32, name="g16")
        nc.gpsimd.dma_start(g16[:], cc_out[:])

        # ---- scales: s = 224/d via reciprocal + one Newton step ----
        MUL = mybir.AluOpType.mult
        SUB = mybir.AluOpType.subtract
        d2 = small.tile([1, 2], f32, name="d2")
        nc.vector.tensor_copy(d2[:], g16[:, 0:2])
        for r in range(1, n_cores):
            nc.vector.tensor_tensor(d2[:], d2[:], g16[:, 2 * r : 2 * r + 2], MAX)
        nc.vector.tensor_scalar_max(d2[:], d2[:], 1e-12)
        r2 = small.tile([1, 2], f32, name="r2")
        nc.vector.reciprocal(r2[:], d2[:])
        y0 = small.tile([1, 2], f32, name="y0")
        nc.vector.tensor_scalar_mul(y0[:], r2[:], FP8_HW_MAX)
        t2 = small.tile([1, 2], f32, name="t2")
        nc.vector.tensor_tensor(t2[:], y0[:], d2[:], MUL)
        nc.vector.tensor_scalar_sub(t2[:], t2[:], FP8_HW_MAX)  # y0*d - 224
        nc.vector.tensor_tensor(t2[:], t2[:], r2[:], MUL)
        s2 = small.tile([1, 2], f32, name="s2")
        nc.vector.tensor_tensor(s2[:], y0[:], t2[:], SUB)      # y0 - r*(y0*d-224)

        def bcast(src11, name):
            b = const.tile([P, 1], f32, name=name)
            nc.gpsimd.partition_broadcast(b[:], src11)
            return b

        sk_b = bcast(s2[:, 0:1], "sk_b")
        sx_b = bcast(s2[:, 1:2], "sx_b")
        inv1 = small.tile([1, 1], f32, name="inv1")
        nc.vector.tensor_tensor(inv1[:], d2[:, 0:1], d2[:, 1:2], MUL)
        nc.vector.tensor_scalar_mul(inv1[:], inv1[:], 1.0 / (FP8_HW_MAX * FP8_HW_MAX))
        inv_b = bcast(inv1[:], "inv_b")

        # ---- resident fp8 kernel shard [P, 32, 2048] ----
        kq = kqp.tile([P, KO_N, US], fp8, name="kq")

        # ---- x stream (ring A) and fused ACT quantize, separable ----
        xst_tiles = {}
        xq_tiles = {}
        last_stream = {"d": last_xscan}

        def stream(b):
            sts = []
            t0 = b * BLK
            for g in range(8):
                st = xpool.tile([P, 4, BLK], f32, tag="xst", name=f"xst{b}_{g}")
                dma = nc.sync.dma_start(
                    st[:], xTr[:, 4 * g : 4 * g + 4, t0 : t0 + BLK]
                )
                if g == 0:
                    add_dep_helper(dma.ins, last_stream["d"].ins, sync=True,
                                   reason="x blocks stream in consumption order")
                sts.append(st)
            last_stream["d"] = dma
            xst_tiles[b] = sts

        def quant(b):
            xq_t = xqp.tile([P, KO_N, BLK], fp8, tag="xq", name=f"xq{b}")
            for g, st in enumerate(xst_tiles.pop(b)):
                nc.scalar.activation(
                    xq_t[:, 4 * g : 4 * g + 4, :], st[:], COPY, scale=sx_b[:]
                )
            xq_tiles[b] = xq_t

        def restream_half(h, eng):
            """k restream + fused DVE quantize for unit columns [h*1024, ...)."""
            for k in range(NPAIR):
                st = kst.tile([P, 2, 1024], f32, tag="kst", name=f"rs{h}_{k}")
                dma = eng.dma_start(
                    st[:], kshr[:, 2 * k : 2 * k + 2, h * 1024 : (h + 1) * 1024]
                )
                if h == 0 and k == 0:
                    add_dep_helper(dma.ins, last_stream["d"].ins, sync=True,
                                   reason="restream h1 behind block-0 stream")
                nc.vector.tensor_scalar_mul(
                    kq[:, 2 * k : 2 * k + 2, h * 1024 : (h + 1) * 1024],
                    st[:], sk_b[:],
                )
                last_stream["d"] = dma

        # ring A: block-0 stream, restream half 1, block-1 stream.
        stream(0)
        restream_half(0, nc.sync)
        stream(1)
        # ACT queue: block-0 quants first, THEN the ring-B (scalar-issued)
        # restream of half 2 -- its issues may stall on kst recycle but only
        # after xq0 is already quantized.
        quant(0)
        restream_half(1, nc.scalar)

        # ---- matmuls + epilogue ----
        def epilogue(b, ub, pt):
            ot = outp.tile([P, BLK], f16, tag="ot", name="ot")
            nc.scalar.activation(
                ot[:], pt[:], GELU, bias=bias_t[:, ub : ub + 1], scale=inv_b[:]
            )
            nc.scalar.dma_start(
                out[ub * P : (ub + 1) * P, b * BLK : (b + 1) * BLK], ot[:]
            )

        def mm_group(xq_t, ubs, blk_psums):
            for kk in range(NPAIR):
                for j, ub in enumerate(ubs):
                    nc.tensor.matmul(
                        blk_psums[j][:],
                        kq[:, 2 * kk : 2 * kk + 2, ub * P : (ub + 1) * P],
                        xq_t[:, 2 * kk : 2 * kk + 2, :],
                        start=(kk == 0), stop=(kk == NPAIR - 1),
                        perf_mode=DR,
                    )

        # block 0: kk-ordered, two 8-ub groups aligned with restream halves;
        # block-1 quantize slotted between the two epilogue groups.
        xq0 = xq_tiles.pop(0)
        for grp in range(2):
            pts = [
                psum.tile([P, BLK], f32, tag="ps", name=f"b0p{grp}_{j}")
                for j in range(8)
            ]
            mm_group(xq0, range(grp * 8, grp * 8 + 8), pts)
            for j in range(8):
                epilogue(0, grp * 8 + j, pts[j])
            if grp == 0:
                quant(1)
        stream(2)
        quant(2)

        for b in range(1, NBLK):
            xq_t = xq_tiles.pop(b)
            for ub in range(NU):
                pt = psum.tile([P, BLK], f32, tag="ps", name=f"ps{ub}")
                for kk in range(NPAIR):
                    nc.tensor.matmul(
                        pt[:],
                        kq[:, 2 * kk : 2 * kk + 2, ub * P : (ub + 1) * P],
                        xq_t[:, 2 * kk : 2 * kk + 2, :],
                        start=(kk == 0), stop=(kk == NPAIR - 1),
                        perf_mode=DR,
                    )
                epilogue(b, ub, pt)
            if b + 2 < NBLK:
                stream(b + 2)
                quant(b + 2)

    nc.compile()
    return nc


def make_in_maps(x, kern, bias, n_cores=N_CORES):
    xT = np.ascontiguousarray(x.T)
    us = kern.shape[1] // n_cores
    amx_t = x.shape[0] // n_cores
    in_maps = []
    for c in range(n_cores):
        in_maps.append(
            {
                "xT": xT,
                "xsl": np.ascontiguousarray(xT[:, c * amx_t : (c + 1) * amx_t]),
                "ksh": np.ascontiguousarray(kern[:, c * us : (c + 1) * us]),
                "bsh": np.ascontiguousarray(bias[c * us : (c + 1) * us]),
            }
        )
    return in_maps


_CACHE = {}


def _built():
    if "nc" not in _CACHE:
        _CACHE["nc"] = build()
    return _CACHE["nc"]


def run(x, kern, bias, trace=False, **kwargs):
    """Run on hardware; returns (full_output, BassKernelResults)."""
    nc = _built()
    in_maps = make_in_maps(x, kern, bias)
    res = run_bass_kernel_spmd(
        nc, in_maps, core_ids=list(range(N_CORES)), trace=trace, **kwargs
    )
    shards = [res.results[c]["out"] for c in range(N_CORES)]
    full = np.concatenate(shards, axis=0)  # [units, tokens] f16
    return full.T.astype(np.float32), res


def kernel(x, kernel, bias):
    out, _ = run(
        np.ascontiguousarray(x, dtype=np.float32),
        np.ascontiguousarray(kernel, dtype=np.float32),
        np.ascontiguousarray(bias, dtype=np.float32),
    )
    return out


# revision 15
# speedup vs baseline: 1.0079x; 1.0079x over previous
"""Trainium2 Bass kernel: fp8-quantized Dense (8192x4096 @ 4096x16384) + bias + tanh-GELU.

Strategy (tensor-parallel over units, 8 cores), v2:
  - host: transpose x -> xT [d_in, tokens]; shard kernel/bias along units.
  - device per core:
      phase 1: amax scans (k shard 32 MiB, then this core's 1/8 token slice of
               xT 16 MiB) in 1 MiB chunks on both HWDGE rings; one
               AllReduce(max) carries [amax_k, amax_x]. CC input/readback DMAs
               ride the gpsimd SWDGE queue so they never block the rings.
      phase 2: scales via reciprocal + one Newton step (~1 ulp; the exact-RNE
               division of v1 isn't needed at the 2e-2 gate). Fused one-op
               quantizes: k on DVE (tensor_scalar mul -> fp8), x on ACT
               (activation Copy w/ scale -> fp8).
      phase 3: k shard restreamed as [128,2,1024] column-half chunks, units
               0..1023 first: block 0 runs kk-ordered in two 8-ub PSUM groups
               so its matmuls consume chunks in exactly the restream order --
               the PE starts ~195 us in and the restream hides behind it.
               x streams in 512-token blocks (f32, quantized on arrival into
               a double-buffered fp8 xq), ub-ordered DoubleRow matmuls,
               ACT epilogue gelu_tanh(psum * inv_scale + bias) -> f16 out.
  - fp8 numerics: quantize with 224/amax (half the reference's 448/amax, a
    power-of-two ratio) so the TRN fp8e4 grid matches OCP e4m3fn exactly in
    [-240, 240]; dequant amax_x*amax_k/224^2 restores the reference values.
  - output is produced transposed ([units, tokens] f16 per core); the host
    gathers shards, transposes, and upcasts to f32.
"""

import sys

sys.path.insert(0, "/opt/trn_rl_repo")

from contextlib import ExitStack

import numpy as np

import concourse.bacc as bacc
import concourse.tile as tile
from concourse import mybir
from concourse.bass_utils import run_bass_kernel_spmd

P = 128
FP8_HW_MAX = 224.0  # 448/2: keeps hw fp8 values inside TRN's +/-240 range

TOKENS, D_IN, UNITS, N_CORES = 8192, 4096, 16384, 8
US = UNITS // N_CORES          # 2048 units per core
KO_N = D_IN // P               # 32 d_in slabs
NPAIR = KO_N // 2              # 16 DoubleRow (256-contraction) steps
NU = US // P                   # 16 output unit-blocks
BLK = 512                      # token block
NBLK = TOKENS // BLK           # 16
AMX_T = TOKENS // N_CORES      # 1024 tokens scanned per core


def build(n_cores=N_CORES):
    dt = mybir.dt
    f32 = dt.float32
    f16 = dt.float16
    fp8 = dt.float8e4
    X = mybir.AxisListType.X
    MAX = mybir.AluOpType.max
    COPY = mybir.ActivationFunctionType.Copy
    GELU = mybir.ActivationFunctionType.Gelu_apprx_tanh
    DR = mybir.MatmulPerfMode.DoubleRow

    nc = bacc.Bacc("TRN2", target_bir_lowering=False, debug=False, num_devices=n_cores)
    xT = nc.dram_tensor("xT", [D_IN, TOKENS], f32, kind="ExternalInput").ap()
    xsl = nc.dram_tensor("xsl", [D_IN, AMX_T], f32, kind="ExternalInput").ap()
    ksh = nc.dram_tensor("ksh", [D_IN, US], f32, kind="ExternalInput").ap()
    bsh = nc.dram_tensor("bsh", [US], f32, kind="ExternalInput").ap()
    out = nc.dram_tensor("out", [US, TOKENS], f16, kind="ExternalOutput").ap()

    xTr = xT.rearrange("(n p) t -> p n t", p=P)    # [128, 32, 8192]
    xslr = xsl.rearrange("(n p) t -> p n t", p=P)  # [128, 32, 1024]
    kshr = ksh.rearrange("(n p) c -> p n c", p=P)  # [128, 32, 2048]

    from concourse.tile_rust import add_dep_helper
    from concourse import bass_isa

    with tile.TileContext(nc) as tc, ExitStack() as ctx:
        const = ctx.enter_context(tc.tile_pool(name="const", bufs=1))
        small = ctx.enter_context(tc.tile_pool(name="small", bufs=1))
        kqp = ctx.enter_context(tc.tile_pool(name="kqp", bufs=1))      # 64 KiB/part
        kst = ctx.enter_context(tc.tile_pool(name="kst", bufs=3))      # 3x8 KiB
        xpool = ctx.enter_context(tc.tile_pool(name="xpool", bufs=8))  # 8x8 KiB
        xqp = ctx.enter_context(tc.tile_pool(name="xqp", bufs=2))      # 2x16 KiB
        outp = ctx.enter_context(tc.tile_pool(name="outp", bufs=4))    # 4x1 KiB
        psum = ctx.enter_context(tc.tile_pool(name="psum", bufs=8, space="PSUM"))
        dram = ctx.enter_context(tc.tile_pool(name="dram", bufs=1, space="DRAM"))

        def ring(i):
            return nc.sync if i % 2 == 0 else nc.scalar

        # ---- bias shard, [P, NU]: bias_t[p, ub] = bias[ub*128 + p] ----
        bias_t = const.tile([P, NU], f32, name="bias_t")
        nc.sync.dma_start(bias_t[:], bsh.rearrange("(o p) -> p o", p=P))

        # ---- amax scans: 1 MiB chunks staged in the (deep) xpool ----
        # k: 32 x [P,2,1024], x-slice: 16 x [P,2,1024], alternating HWDGE rings.
        rk = const.tile([P, KO_N], f32, name="rk")
        last_scan = None
        for i in range(KO_N):
            pr, h = i // 2, i % 2
            st = xpool.tile([P, 2, 1024], f32, tag="xst", name="kscan")
            last_scan = ring(i).dma_start(
                st[:], kshr[:, 2 * pr : 2 * pr + 2, h * 1024 : (h + 1) * 1024]
            )
            nc.vector.tensor_reduce(
                rk[:, i : i + 1], st[:].rearrange("p a b -> p (a b)"), axis=X,
                op=MAX, apply_absolute_value=True,
            )

        rx = const.tile([P, NPAIR], f32, name="rx")
        for i in range(NPAIR):
            st = xpool.tile([P, 2, 1024], f32, tag="xst", name="xscan")
            last_scan = ring(i).dma_start(st[:], xslr[:, 2 * i : 2 * i + 2, :])
            nc.vector.tensor_reduce(
                rx[:, i : i + 1], st[:].rearrange("p a b -> p (a b)"), axis=X,
                op=MAX, apply_absolute_value=True,
            )
        last_xscan = last_scan

        # ---- AllGather of per-core [amax_k, amax_x]; local max-combine ----
        colk = small.tile([P, 1], f32, name="colk")
        nc.vector.tensor_reduce(colk[:], rk[:], axis=X, op=MAX)
        nc.gpsimd.partition_all_reduce(colk[:], colk[:], P, bass_isa.ReduceOp.max)
        colx = small.tile([P, 1], f32, name="colx")
        nc.vector.tensor_reduce(colx[:], rx[:], axis=X, op=MAX)
        nc.gpsimd.partition_all_reduce(colx[:], colx[:], P, bass_isa.ReduceOp.max)

        pk2 = small.tile([1, 2], f32, name="pk2")
        nc.vector.tensor_copy(pk2[:, 0:1], colk[0:1, :])
        nc.vector.tensor_copy(pk2[:, 1:2], colx[0:1, :])
        cc_in = dram.tile([1, 2], f32, name="cc_in")
        nc.gpsimd.dma_start(cc_in[:], pk2[:])
        cc_out = dram.tile([1, 2 * n_cores], f32, name="cc_out", addr_space="Shared")
        nc.gpsimd.collective_compute(
            "AllGather", mybir.AluOpType.bypass,
            replica_groups=[list(range(n_cores))],
            ins=[cc_in[:].opt()], outs=[cc_out[:].opt()],
        )
        g16 = small.tile([1, 2 * n_cores], f32, name="g16")
        nc.gpsimd.dma_start(g16[:], cc_out[:])

        # ---- scales: s = 224/d via reciprocal + one Newton step ----
        MUL = mybir.AluOpType.mult
        SUB = mybir.AluOpType.subtract
        d2 = small.tile([1, 2], f32, name="d2")
        nc.vector.tensor_copy(d2[:], g16[:, 0:2])
        for r in range(1, n_cores):
            nc.vector.tensor_tensor(d2[:], d2[:], g16[:, 2 * r : 2 * r + 2], MAX)
        nc.vector.tensor_scalar_max(d2[:], d2[:], 1e-12)
        r2 = small.tile([1, 2], f32, name="r2")
        nc.vector.reciprocal(r2[:], d2[:])
        y0 = small.tile([1, 2], f32, name="y0")
        nc.vector.tensor_scalar_mul(y0[:], r2[:], FP8_HW_MAX)
        t2 = small.tile([1, 2], f32, name="t2")
        nc.vector.tensor_tensor(t2[:], y0[:], d2[:], MUL)
        nc.vector.tensor_scalar_sub(t2[:], t2[:], FP8_HW_MAX)  # y0*d - 224
        nc.vector.tensor_tensor(t2[:], t2[:], r2[:], MUL)
        s2 = small.tile([1, 2], f32, name="s2")
        nc.vector.tensor_tensor(s2[:], y0[:], t2[:], SUB)      # y0 - r*(y0*d-224)

        def bcast(src11, name):
            b = const.tile([P, 1], f32, name=name)
            nc.gpsimd.partition_broadcast(b[:], src11)
            return b

        sk_b = bcast(s2[:, 0:1], "sk_b")
        sx_b = bcast(s2[:, 1:2], "sx_b")
        inv1 = small.tile([1, 1], f32, name="inv1")
        nc.vector.tensor_tensor(inv1[:], d2[:, 0:1], d2[:, 1:2], MUL)
        nc.vector.tensor_scalar_mul(inv1[:], inv1[:], 1.0 / (FP8_HW_MAX * FP8_HW_MAX))
        inv_b = bcast(inv1[:], "inv_b")

        # ---- resident fp8 kernel shard [P, 32, 2048] ----
        kq = kqp.tile([P, KO_N, US], fp8, name="kq")

        # ---- x stream (ring A) and fused ACT quantize, separable ----
        xst_tiles = {}
        xq_tiles = {}
        last_stream = {"d": last_xscan}

        def stream(b):
            sts = []
            t0 = b * BLK
            for g in range(8):
                st = xpool.tile([P, 4, BLK], f32, tag="xst", name=f"xst{b}_{g}")
                dma = nc.sync.dma_start(
                    st[:], xTr[:, 4 * g : 4 * g + 4, t0 : t0 + BLK]
                )
                if g == 0:
                    add_dep_helper(dma.ins, last_stream["d"].ins, sync=True,
                                   reason="x blocks stream in consumption order")
                sts.append(st)
            last_stream["d"] = dma
            xst_tiles[b] = sts

        def quant(b):
            if b == 0:
                xq_t = const.tile([P, KO_N, BLK], fp8, name="xq0")  # pinned
            else:
                xq_t = xqp.tile([P, KO_N, BLK], fp8, tag="xq", name=f"xq{b}")
            for g, st in enumerate(xst_tiles.pop(b)):
                nc.scalar.activation(
                    xq_t[:, 4 * g : 4 * g + 4, :], st[:], COPY, scale=sx_b[:]
                )
            xq_tiles[b] = xq_t

        # ---- k restream (all on ring A / sync), half 1 then half 2 ----
        # Half-1 pairs 0..3 ride in the freed scan-staging slots during the
        # collective's latency window; the rest stage through kst. Half 2 is
        # consumed only by the DEFERRED block-0 group B at the very end, so
        # its delivery pace is irrelevant.
        stream(0)

        def rs_chain(dma, why):
            add_dep_helper(dma.ins, last_stream["d"].ins, sync=True, reason=why)
            last_stream["d"] = dma

        for e in range(2):  # h1 pairs 0..3 banked in xq-tag staging
            st = xqp.tile([P, 4, 1024], f32, tag="xq", name=f"rsbank{e}")
            dma = nc.sync.dma_start(st[:], kshr[:, 4 * e : 4 * e + 4, 0:1024])
            if e == 0:
                rs_chain(dma, "restream bank behind block-0 stream")
            else:
                last_stream["d"] = dma
            nc.vector.tensor_scalar_mul(
                kq[:, 4 * e : 4 * e + 4, 0:1024], st[:], sk_b[:]
            )

        def restream(h, pairs):
            for k in pairs:
                st = kst.tile([P, 2, 1024], f32, tag="kst", name=f"rs{h}_{k}")
                dma = nc.sync.dma_start(
                    st[:], kshr[:, 2 * k : 2 * k + 2, h * 1024 : (h + 1) * 1024]
                )
                last_stream["d"] = dma
                nc.vector.tensor_scalar_mul(
                    kq[:, 2 * k : 2 * k + 2, h * 1024 : (h + 1) * 1024],
                    st[:], sk_b[:],
                )

        restream(0, range(4, NPAIR))
        stream(1)
        restream(1, range(NPAIR))
        quant(0)

        # ---- matmuls + epilogue ----
        def epilogue(b, ub, pt):
            ot = outp.tile([P, BLK], f16, tag="ot", name="ot")
            nc.scalar.activation(
                ot[:], pt[:], GELU, bias=bias_t[:, ub : ub + 1], scale=inv_b[:]
            )
            nc.scalar.dma_start(
                out[ub * P : (ub + 1) * P, b * BLK : (b + 1) * BLK], ot[:]
            )

        def mm_group(xq_t, ubs, blk_psums):
            for kk in range(NPAIR):
                for j, ub in enumerate(ubs):
                    nc.tensor.matmul(
                        blk_psums[j][:],
                        kq[:, 2 * kk : 2 * kk + 2, ub * P : (ub + 1) * P],
                        xq_t[:, 2 * kk : 2 * kk + 2, :],
                        start=(kk == 0), stop=(kk == NPAIR - 1),
                        perf_mode=DR,
                    )

        # block 0, group A only (units 0..1023 = restream half 1), kk-ordered
        # so it consumes half-1 pairs in delivery order. Group B (units
        # 1024..2047) is DEFERRED to the very end, when half 2 is resident.
        xq0 = xq_tiles.pop(0)
        pts = [psum.tile([P, BLK], f32, tag="ps", name=f"b0a{j}") for j in range(8)]
        mm_group(xq0, range(8), pts)
        for j in range(8):
            epilogue(0, j, pts[j])
        quant(1)
        stream(2)
        quant(2)

        for b in range(1, NBLK):
            xq_t = xq_tiles.pop(b)
            for ub in range(NU):
                pt = psum.tile([P, BLK], f32, tag="ps", name=f"ps{ub}")
                for kk in range(NPAIR):
                    nc.tensor.matmul(
                        pt[:],
                        kq[:, 2 * kk : 2 * kk + 2, ub * P : (ub + 1) * P],
                        xq_t[:, 2 * kk : 2 * kk + 2, :],
                        start=(kk == 0), stop=(kk == NPAIR - 1),
                        perf_mode=DR,
                    )
                epilogue(b, ub, pt)
            if b + 2 < NBLK:
                stream(b + 2)
                quant(b + 2)

        # deferred block-0 group B (units 1024..2047)
        pts = [psum.tile([P, BLK], f32, tag="ps", name=f"b0b{j}") for j in range(8)]
        mm_group(xq0, range(8, NU), pts)
        for j in range(8):
            epilogue(0, 8 + j, pts[j])

    nc.compile()
    return nc


def make_in_maps(x, kern, bias, n_cores=N_CORES):
    xT = np.ascontiguousarray(x.T)
    us = kern.shape[1] // n_cores
    amx_t = x.shape[0] // n_cores
    in_maps = []
    for c in range(n_cores):
        in_maps.append(
            {
                "xT": xT,
                "xsl": np.ascontiguousarray(xT[:, c * amx_t : (c + 1) * amx_t]),
                "ksh": np.ascontiguousarray(kern[:, c * us : (c + 1) * us]),
                "bsh": np.ascontiguousarray(bias[c * us : (c + 1) * us]),
            }
        )
    return in_maps


_CACHE = {}


def _built():
    if "nc" not in _CACHE:
        _CACHE["nc"] = build()
    return _CACHE["nc"]


def run(x, kern, bias, trace=False, **kwargs):
    """Run on hardware; returns (full_output, BassKernelResults)."""
    nc = _built()
    in_maps = make_in_maps(x, kern, bias)
    res = run_bass_kernel_spmd(
        nc, in_maps, core_ids=list(range(N_CORES)), trace=trace, **kwargs
    )
    shards = [res.results[c]["out"] for c in range(N_CORES)]
    full = np.concatenate(shards, axis=0)  # [units, tokens] f16
    return full.T.astype(np.float32), res


def kernel(x, kernel, bias):
    out, _ = run(
        np.ascontiguousarray(x, dtype=np.float32),
        np.ascontiguousarray(kernel, dtype=np.float32),
        np.ascontiguousarray(bias, dtype=np.float32),
    )
    return out


# revision 19
# speedup vs baseline: 1.0337x; 1.0255x over previous
"""Trainium2 Bass kernel: fp8-quantized Dense (8192x4096 @ 4096x16384) + bias + tanh-GELU.

Strategy (tensor-parallel over units, 8 cores), v2:
  - host: transpose x -> xT [d_in, tokens]; shard kernel/bias along units.
  - device per core:
      phase 1: amax scans (k shard 32 MiB, then this core's 1/8 token slice of
               xT 16 MiB) in 1 MiB chunks on both HWDGE rings; one
               AllReduce(max) carries [amax_k, amax_x]. CC input/readback DMAs
               ride the gpsimd SWDGE queue so they never block the rings.
      phase 2: scales via reciprocal + one Newton step (~1 ulp; the exact-RNE
               division of v1 isn't needed at the 2e-2 gate). Fused one-op
               quantizes: k on DVE (tensor_scalar mul -> fp8), x on ACT
               (activation Copy w/ scale -> fp8).
      phase 3: k shard restreamed as [128,2,1024] column-half chunks, units
               0..1023 first: block 0 runs kk-ordered in two 8-ub PSUM groups
               so its matmuls consume chunks in exactly the restream order --
               the PE starts ~195 us in and the restream hides behind it.
               x streams in 512-token blocks (f32, quantized on arrival into
               a double-buffered fp8 xq), ub-ordered DoubleRow matmuls,
               ACT epilogue gelu_tanh(psum * inv_scale + bias) -> f16 out.
  - fp8 numerics: quantize with 224/amax (half the reference's 448/amax, a
    power-of-two ratio) so the TRN fp8e4 grid matches OCP e4m3fn exactly in
    [-240, 240]; dequant amax_x*amax_k/224^2 restores the reference values.
  - output is produced transposed ([units, tokens] f16 per core); the host
    gathers shards, transposes, and upcasts to f32.
"""

import sys

sys.path.insert(0, "/opt/trn_rl_repo")

from contextlib import ExitStack

import numpy as np

import concourse.bacc as bacc
import concourse.tile as tile
from concourse import mybir
from concourse.bass_utils import run_bass_kernel_spmd

P = 128
FP8_HW_MAX = 224.0  # 448/2: keeps hw fp8 values inside TRN's +/-240 range

TOKENS, D_IN, UNITS, N_CORES = 8192, 4096, 16384, 8
US = UNITS // N_CORES          # 2048 units per core
KO_N = D_IN // P               # 32 d_in slabs
NPAIR = KO_N // 2              # 16 DoubleRow (256-contraction) steps
NU = US // P                   # 16 output unit-blocks
BLK = 512                      # token block
NBLK = TOKENS // BLK           # 16
AMX_T = TOKENS // N_CORES      # 1024 tokens scanned per core


def build(n_cores=N_CORES):
    dt = mybir.dt
    f32 = dt.float32
    f16 = dt.float16
    fp8 = dt.float8e4
    X = mybir.AxisListType.X
    MAX = mybir.AluOpType.max
    COPY = mybir.ActivationFunctionType.Copy
    GELU = mybir.ActivationFunctionType.Gelu_apprx_tanh
    DR = mybir.MatmulPerfMode.DoubleRow

    nc = bacc.Bacc("TRN2", target_bir_lowering=False, debug=False, num_devices=n_cores)
    xT = nc.dram_tensor("xT", [D_IN, TOKENS], f32, kind="ExternalInput").ap()
    xsl = nc.dram_tensor("xsl", [D_IN, AMX_T], f32, kind="ExternalInput").ap()
    ksh = nc.dram_tensor("ksh", [D_IN, US], f32, kind="ExternalInput").ap()
    bsh = nc.dram_tensor("bsh", [US], f32, kind="ExternalInput").ap()
    out = nc.dram_tensor("out", [US, TOKENS], f16, kind="ExternalOutput").ap()

    xTr = xT.rearrange("(n p) t -> p n t", p=P)    # [128, 32, 8192]
    xslr = xsl.rearrange("(n p) t -> p n t", p=P)  # [128, 32, 1024]
    kshr = ksh.rearrange("(n p) c -> p n c", p=P)  # [128, 32, 2048]

    from concourse.tile_rust import add_dep_helper
    from concourse import bass_isa

    with tile.TileContext(nc) as tc, ExitStack() as ctx:
        const = ctx.enter_context(tc.tile_pool(name="const", bufs=1))
        small = ctx.enter_context(tc.tile_pool(name="small", bufs=1))
        kqp = ctx.enter_context(tc.tile_pool(name="kqp", bufs=1))      # 64 KiB/part
        xpool = ctx.enter_context(tc.tile_pool(name="xpool", bufs=8))  # 8x8 KiB
        xqp = ctx.enter_context(tc.tile_pool(name="xqp", bufs=3))      # 3x16 KiB
        outp = ctx.enter_context(tc.tile_pool(name="outp", bufs=4))    # 4x1 KiB
        psum = ctx.enter_context(tc.tile_pool(name="psum", bufs=8, space="PSUM"))
        dram = ctx.enter_context(tc.tile_pool(name="dram", bufs=1, space="DRAM"))

        def ring(i):
            return nc.sync if i % 2 == 0 else nc.scalar

        # ---- bias shard, [P, NU]: bias_t[p, ub] = bias[ub*128 + p] ----
        bias_t = const.tile([P, NU], f32, name="bias_t")
        nc.sync.dma_start(bias_t[:], bsh.rearrange("(o p) -> p o", p=P))

        # ---- amax scans: 2 MiB chunks staged in the (idle) xq pool ----
        # k: 16 x [P,2,2048], x-slice: 8 x [P,4,1024], alternating HWDGE rings.
        rk = const.tile([P, NPAIR], f32, name="rk")
        last_scan = None
        for i in range(NPAIR):
            st = xqp.tile([P, 2, 2048], f32, tag="xq", name="kscan")
            last_scan = ring(i).dma_start(st[:], kshr[:, 2 * i : 2 * i + 2, :])
            nc.vector.tensor_reduce(
                rk[:, i : i + 1], st[:].rearrange("p a b -> p (a b)"), axis=X,
                op=MAX, apply_absolute_value=True,
            )

        rx = const.tile([P, 8], f32, name="rx")
        for i in range(8):
            st = xqp.tile([P, 4, 1024], f32, tag="xq", name="xscan")
            last_scan = ring(i).dma_start(st[:], xslr[:, 4 * i : 4 * i + 4, :])
            nc.vector.tensor_reduce(
                rx[:, i : i + 1], st[:].rearrange("p a b -> p (a b)"), axis=X,
                op=MAX, apply_absolute_value=True,
            )
        last_xscan = last_scan

        # ---- AllGather of per-core [amax_k, amax_x]; local max-combine ----
        colk = small.tile([P, 1], f32, name="colk")
        nc.vector.tensor_reduce(colk[:], rk[:], axis=X, op=MAX)
        nc.gpsimd.partition_all_reduce(colk[:], colk[:], P, bass_isa.ReduceOp.max)
        colx = small.tile([P, 1], f32, name="colx")
        nc.vector.tensor_reduce(colx[:], rx[:], axis=X, op=MAX)
        nc.gpsimd.partition_all_reduce(colx[:], colx[:], P, bass_isa.ReduceOp.max)

        pk2 = small.tile([1, 2], f32, name="pk2")
        nc.vector.tensor_copy(pk2[:, 0:1], colk[0:1, :])
        nc.vector.tensor_copy(pk2[:, 1:2], colx[0:1, :])
        cc_in = dram.tile([1, 2], f32, name="cc_in")
        nc.gpsimd.dma_start(cc_in[:], pk2[:])
        cc_out = dram.tile([1, 2 * n_cores], f32, name="cc_out", addr_space="Shared")
        nc.gpsimd.collective_compute(
            "AllGather", mybir.AluOpType.bypass,
            replica_groups=[list(range(n_cores))],
            ins=[cc_in[:].opt()], outs=[cc_out[:].opt()],
        )
        g16 = small.tile([1, 2 * n_cores], f32, name="g16")
        nc.gpsimd.dma_start(g16[:], cc_out[:])

        # ---- scales: s = 224/d via reciprocal + one Newton step ----
        MUL = mybir.AluOpType.mult
        SUB = mybir.AluOpType.subtract
        d2 = small.tile([1, 2], f32, name="d2")
        nc.vector.tensor_copy(d2[:], g16[:, 0:2])
        for r in range(1, n_cores):
            nc.vector.tensor_tensor(d2[:], d2[:], g16[:, 2 * r : 2 * r + 2], MAX)
        nc.vector.tensor_scalar_max(d2[:], d2[:], 1e-12)
        r2 = small.tile([1, 2], f32, name="r2")
        nc.vector.reciprocal(r2[:], d2[:])
        y0 = small.tile([1, 2], f32, name="y0")
        nc.vector.tensor_scalar_mul(y0[:], r2[:], FP8_HW_MAX)
        t2 = small.tile([1, 2], f32, name="t2")
        nc.vector.tensor_tensor(t2[:], y0[:], d2[:], MUL)
        nc.vector.tensor_scalar_sub(t2[:], t2[:], FP8_HW_MAX)  # y0*d - 224
        nc.vector.tensor_tensor(t2[:], t2[:], r2[:], MUL)
        s2 = small.tile([1, 2], f32, name="s2")
        nc.vector.tensor_tensor(s2[:], y0[:], t2[:], SUB)      # y0 - r*(y0*d-224)

        def bcast(src11, name):
            b = const.tile([P, 1], f32, name=name)
            nc.gpsimd.partition_broadcast(b[:], src11)
            return b

        sk_b = bcast(s2[:, 0:1], "sk_b")
        sx_b = bcast(s2[:, 1:2], "sx_b")
        inv1 = small.tile([1, 1], f32, name="inv1")
        nc.vector.tensor_tensor(inv1[:], d2[:, 0:1], d2[:, 1:2], MUL)
        nc.vector.tensor_scalar_mul(inv1[:], inv1[:], 1.0 / (FP8_HW_MAX * FP8_HW_MAX))
        inv_b = bcast(inv1[:], "inv_b")

        # ---- resident fp8 kernel shard [P, 32, 2048] ----
        kq = kqp.tile([P, KO_N, US], fp8, name="kq")

        # ---- x stream (ring A) and fused ACT quantize, separable ----
        xst_tiles = {}
        xq_tiles = {}
        last_stream = {"d": last_xscan}

        def stream(b):
            sts = []
            t0 = b * BLK
            for g in range(8):
                st = xpool.tile([P, 4, BLK], f32, tag="xst", name=f"xst{b}_{g}")
                dma = nc.sync.dma_start(
                    st[:], xTr[:, 4 * g : 4 * g + 4, t0 : t0 + BLK]
                )
                if g == 0:
                    add_dep_helper(dma.ins, last_stream["d"].ins, sync=True,
                                   reason="x blocks stream in consumption order")
                sts.append(st)
            last_stream["d"] = dma
            xst_tiles[b] = sts

        def quant(b, eng=None):
            if b == 0:
                xq_t = const.tile([P, KO_N, BLK], fp8, name="xq0")  # pinned
            else:
                xq_t = xqp.tile([P, KO_N, BLK], fp8, tag="xq", name=f"xq{b}")
            for g, st in enumerate(xst_tiles.pop(b)):
                if eng is None:
                    # fused mul+fp8-convert on the (steady-state idle) DVE
                    nc.vector.tensor_scalar_mul(
                        xq_t[:, 4 * g : 4 * g + 4, :], st[:], sx_b[:]
                    )
                else:
                    eng.activation(
                        xq_t[:, 4 * g : 4 * g + 4, :], st[:], COPY, scale=sx_b[:]
                    )
            xq_tiles[b] = xq_t

        # ---- k restream (all on ring A / sync), half 1 then half 2 ----
        # Half-1 pairs 0..5 ride in the freed scan-staging slots during the
        # collective's latency window; the rest stage through the xst tag.
        # Half 2 is consumed only by the DEFERRED block-0 group B at the very
        # end, so its delivery pace is irrelevant.
        stream(0)

        for e in range(3):  # h1 pairs 0..5 banked in xq-tag staging
            st = xqp.tile([P, 4, 1024], f32, tag="xq", name=f"rsbank{e}")
            dma = nc.sync.dma_start(st[:], kshr[:, 4 * e : 4 * e + 4, 0:1024])
            if e == 0:
                add_dep_helper(dma.ins, last_stream["d"].ins, sync=True,
                               reason="restream bank behind block-0 stream")
            last_stream["d"] = dma
            nc.vector.tensor_scalar_mul(
                kq[:, 4 * e : 4 * e + 4, 0:1024], st[:], sk_b[:]
            )

        def restream(h, pairs):
            for k in pairs:
                st = xpool.tile([P, 2, 1024], f32, tag="xst", name=f"rs{h}_{k}")
                dma = nc.sync.dma_start(
                    st[:], kshr[:, 2 * k : 2 * k + 2, h * 1024 : (h + 1) * 1024]
                )
                last_stream["d"] = dma
                nc.vector.tensor_scalar_mul(
                    kq[:, 2 * k : 2 * k + 2, h * 1024 : (h + 1) * 1024],
                    st[:], sk_b[:],
                )

        restream(0, range(6, NPAIR))
        stream(1)
        restream(1, range(NPAIR))
        quant(0, eng=nc.scalar)

        # ---- matmuls + epilogue ----
        def epilogue(b, ub, pt):
            ot = outp.tile([P, BLK], f16, tag="ot", name="ot")
            nc.scalar.activation(
                ot[:], pt[:], GELU, bias=bias_t[:, ub : ub + 1], scale=inv_b[:]
            )
            nc.scalar.dma_start(
                out[ub * P : (ub + 1) * P, b * BLK : (b + 1) * BLK], ot[:]
            )

        def mm_group(xq_t, ubs, blk_psums):
            for kk in range(NPAIR):
                for j, ub in enumerate(ubs):
                    nc.tensor.matmul(
                        blk_psums[j][:],
                        kq[:, 2 * kk : 2 * kk + 2, ub * P : (ub + 1) * P],
                        xq_t[:, 2 * kk : 2 * kk + 2, :],
                        start=(kk == 0), stop=(kk == NPAIR - 1),
                        perf_mode=DR,
                    )

        # block 0, group A only (units 0..1023 = restream half 1), kk-ordered
        # so it consumes half-1 pairs in delivery order. Group B (units
        # 1024..2047) is DEFERRED to the very end, when half 2 is resident.
        xq0 = xq_tiles.pop(0)
        pts = [psum.tile([P, BLK], f32, tag="ps", name=f"b0a{j}") for j in range(8)]
        mm_group(xq0, range(8), pts)
        quant(1, eng=nc.scalar)
        for j in range(8):
            epilogue(0, j, pts[j])
        stream(2)
        quant(2, eng=nc.scalar)

        for b in range(1, NBLK):
            xq_t = xq_tiles.pop(b)
            for ub in range(NU):
                pt = psum.tile([P, BLK], f32, tag="ps", name=f"ps{ub}")
                for kk in range(NPAIR):
                    nc.tensor.matmul(
                        pt[:],
                        kq[:, 2 * kk : 2 * kk + 2, ub * P : (ub + 1) * P],
                        xq_t[:, 2 * kk : 2 * kk + 2, :],
                        start=(kk == 0), stop=(kk == NPAIR - 1),
                        perf_mode=DR,
                    )
                epilogue(b, ub, pt)
            if b + 2 < NBLK:
                stream(b + 2)
                quant(b + 2)

        # deferred block-0 group B (units 1024..2047), ub-ordered: short tail
        for j in range(8):
            ub = 8 + j
            pt = psum.tile([P, BLK], f32, tag="ps", name=f"b0b{j}")
            for kk in range(NPAIR):
                nc.tensor.matmul(
                    pt[:],
                    kq[:, 2 * kk : 2 * kk + 2, ub * P : (ub + 1) * P],
                    xq0[:, 2 * kk : 2 * kk + 2, :],
                    start=(kk == 0), stop=(kk == NPAIR - 1),
                    perf_mode=DR,
                )
            epilogue(0, ub, pt)

    nc.compile()
    return nc


def make_in_maps(x, kern, bias, n_cores=N_CORES):
    xT = np.ascontiguousarray(x.T)
    us = kern.shape[1] // n_cores
    amx_t = x.shape[0] // n_cores
    in_maps = []
    for c in range(n_cores):
        in_maps.append(
            {
                "xT": xT,
                "xsl": np.ascontiguousarray(xT[:, c * amx_t : (c + 1) * amx_t]),
                "ksh": np.ascontiguousarray(kern[:, c * us : (c + 1) * us]),
                "bsh": np.ascontiguousarray(bias[c * us : (c + 1) * us]),
            }
        )
    return in_maps


_CACHE = {}


def _built():
    if "nc" not in _CACHE:
        _CACHE["nc"] = build()
    return _CACHE["nc"]


def run(x, kern, bias, trace=False, **kwargs):
    """Run on hardware; returns (full_output, BassKernelResults)."""
    nc = _built()
    in_maps = make_in_maps(x, kern, bias)
    res = run_bass_kernel_spmd(
        nc, in_maps, core_ids=list(range(N_CORES)), trace=trace, **kwargs
    )
    shards = [res.results[c]["out"] for c in range(N_CORES)]
    full = np.concatenate(shards, axis=0)  # [units, tokens] f16
    return full.T.astype(np.float32), res


def kernel(x, kernel, bias):
    out, _ = run(
        np.ascontiguousarray(x, dtype=np.float32),
        np.ascontiguousarray(kernel, dtype=np.float32),
        np.ascontiguousarray(bias, dtype=np.float32),
    )
    return out
